# revision 1
# baseline (speedup 1.0000x reference)
"""Trainium2 Bass kernel for nn_LongTermMemoryMLP.

Per-batch-weight 3-layer MLP:
    h0 = relu(q @ W0^T + b0); h1 = relu(h0 @ W1^T + b1); out = h1 @ W2^T + b2
with q: [B,S,DIN], W0: [B,DH,DIN], W1: [B,DH,DH], W2: [B,DOUT,DH], B=8.

Sharding: data-parallel over batch — one batch sample (and its weight slabs)
per NeuronCore, 8 cores, no cross-core communication.

Device-side strategy: activations are kept feature-major ([feature, seq],
feature on partitions) so every layer is a plain accumulated matmul with the
(pre-transposed) weights as the stationary operand and the activations as the
moving operand — no on-chip transposes. The final layer flips orientation
(stationary = activation tile, moving = W2^T) so the output lands seq-major
and can be DMA'd out contiguously. Inputs are pre-transposed on the host.
Matmuls run as float32r (full fp32 storage, PE rounds internally to ~11-12
mantissa bits, streams at 1 row/cycle for N>=256): ~16x more accurate than
bf16 at ~10% more PE time, fp32 accumulation in PSUM.
"""

import numpy as np

import ml_dtypes

import concourse.bass as bass
import concourse.tile as tile
from concourse import bacc, mybir
from concourse.bass_utils import run_bass_kernel_spmd

B, S, DIN, DH, DOUT = 8, 4096, 512, 1024, 512
SC = 512  # seq chunk processed per pipeline iteration

BF16 = mybir.dt.bfloat16
F32 = mybir.dt.float32
F32R = mybir.dt.float32r


def build_nc():
    nc = bacc.Bacc("TRN2")
    qT = nc.dram_tensor("qT", (DIN, S), F32R, kind="ExternalInput")
    w0t = nc.dram_tensor("w0t", (DIN, DH), F32R, kind="ExternalInput")
    w1t = nc.dram_tensor("w1t", (DH, DH), F32R, kind="ExternalInput")
    w2t = nc.dram_tensor("w2t", (DH, DOUT), F32R, kind="ExternalInput")
    b0 = nc.dram_tensor("b0", (DH,), F32, kind="ExternalInput")
    b1 = nc.dram_tensor("b1", (DH,), F32, kind="ExternalInput")
    b2 = nc.dram_tensor("b2", (DOUT,), F32, kind="ExternalInput")
    out = nc.dram_tensor("out", (S, DOUT), F32, kind="ExternalOutput")

    K0 = DIN // 128   # 4  k-tiles, layer 0
    K1 = DH // 128    # 8  k-tiles, layers 1/2
    M0 = DH // 128    # 8  m-tiles (feature tiles of h0/h1)
    MT = SC // 128    # 4  seq m-tiles per chunk, layer 2
    NCH = S // SC     # 8  chunks

    Relu = mybir.ActivationFunctionType.Relu

    with tile.TileContext(nc) as tc:
        with (
            tc.tile_pool(name="weights", bufs=1) as wpool,
            tc.tile_pool(name="biases", bufs=1) as bpool,
            tc.tile_pool(name="acts", bufs=2) as apool,
            tc.tile_pool(name="qin", bufs=2) as qpool,
            tc.tile_pool(name="outp", bufs=4) as opool,
            tc.tile_pool(name="psum0", bufs=2, space="PSUM") as ppool0,
            tc.tile_pool(name="psum1", bufs=3, space="PSUM") as ppool1,
            tc.tile_pool(name="psum2", bufs=3, space="PSUM") as ppool2,
        ):
            # Pre-warm the PE clock gate (HAM) with dummy matmuls on garbage
            # data while the startup DMAs land (the memsets land ~8us in,
            # after the DVE preamble, which matches when the DMA rings go
            # live): the real matmul stream then starts at 2.4 GHz.
            g_lhs = apool.tile([128, 128], BF16, tag="warm_lhs")
            g_rhs = apool.tile([128, SC], BF16, tag="warm_rhs")
            nc.vector.memset(g_lhs, 0.0)
            nc.vector.memset(g_rhs, 0.0)
            warm_ps = ppool0.tile([128, SC], F32, tag="ps0")
            N_WARM = 12
            for i in range(N_WARM):
                nc.tensor.matmul(
                    warm_ps, lhsT=g_lhs, rhs=g_rhs,
                    start=(i == 0), stop=(i == N_WARM - 1),
                )

            # Startup-critical loads: layer-0 weights + the first two seq
            # chunks on the Sync engine's HWDGE ring; W1/W2 go out on the
            # Scalar engine's ring in parallel (one dynamic HWDGE ring per
            # issuing engine, ~150-265 GB/s each, live only after the ~8us
            # engine preamble). W1 is split across both rings so its last
            # tile lands before chunk-0 layer-1 needs it.
            w0_sb = [wpool.tile([128, DH], F32R, tag=f"w0_{k}", name=f"w0_{k}") for k in range(K0)]
            q0_sb = [qpool.tile([128, SC], F32R, tag=f"q_{k}", name=f"q0_{k}") for k in range(K0)]
            for k in range(K0):
                nc.sync.dma_start(out=w0_sb[k], in_=w0t[k * 128:(k + 1) * 128, :])
                nc.sync.dma_start(out=q0_sb[k], in_=qT[k * 128:(k + 1) * 128, 0:SC])
            b0_sb = bpool.tile([128, M0], F32, tag="b0")
            nc.gpsimd.dma_start(out=b0_sb, in_=b0[:].rearrange("(m p) -> p m", p=128))

            q1_sb = []
            for k in range(K0):
                t = qpool.tile([128, SC], F32R, tag=f"q_{k}", name=f"q1pre_{k}")
                nc.sync.dma_start(out=t, in_=qT[k * 128:(k + 1) * 128, SC:2 * SC])
                q1_sb.append(t)

            w1_sb = [wpool.tile([128, DH], F32R, tag=f"w1_{k}", name=f"w1_{k}") for k in range(K1)]
            for k in range(K1):
                eng = nc.sync if k % 2 == 0 else nc.scalar
                eng.dma_start(out=w1_sb[k], in_=w1t[k * 128:(k + 1) * 128, :])
            b1_sb = bpool.tile([128, M0], F32, tag="b1")
            nc.gpsimd.dma_start(out=b1_sb, in_=b1[:].rearrange("(m p) -> p m", p=128))

            w2_sb = [wpool.tile([128, DOUT], F32R, tag=f"w2_{k}", name=f"w2_{k}") for k in range(K1)]
            for k in range(K1):
                nc.scalar.dma_start(out=w2_sb[k], in_=w2t[k * 128:(k + 1) * 128, :])
            b2_sb = bpool.tile([128, DOUT], F32, tag="b2")
            b2_ap = b2[:]
            b2_bcast = bass.AP(
                tensor=b2_ap.tensor,
                offset=b2_ap.offset,
                ap=[[0, 128]] + [list(d) for d in b2_ap.ap],
            )
            nc.gpsimd.dma_start(out=b2_sb, in_=b2_bcast)

            def load_q(c):
                s0 = c * SC
                q_sb = []
                for k in range(K0):
                    t = qpool.tile([128, SC], F32R, tag=f"q_{k}", name=f"q{c}_{k}")
                    nc.sync.dma_start(
                        out=t, in_=qT[k * 128:(k + 1) * 128, s0:s0 + SC]
                    )
                    q_sb.append(t)
                return q_sb

            def layer0(c, q_sb):
                h0_sb = []
                for m in range(M0):
                    ps = ppool0.tile([128, SC], F32, tag="ps0", name=f"ps0_{c}_{m}")
                    for k in range(K0):
                        nc.tensor.matmul(
                            ps,
                            lhsT=w0_sb[k][:, m * 128:(m + 1) * 128],
                            rhs=q_sb[k],
                            start=(k == 0),
                            stop=(k == K0 - 1),
                        )
                    h = apool.tile([128, SC], F32R, tag=f"h0_{m}", name=f"h0_{c}_{m}")
                    nc.scalar.activation(h, ps, Relu, bias=b0_sb[:, m:m + 1])
                    h0_sb.append(h)
                return h0_sb

            def layers12(c, h0_sb):
                s0 = c * SC
                h1_sb = []
                for m in range(M0):
                    ps = ppool1.tile([128, SC], F32, tag="ps1", name=f"ps1_{c}_{m}")
                    for k in range(K1):
                        nc.tensor.matmul(
                            ps,
                            lhsT=w1_sb[k][:, m * 128:(m + 1) * 128],
                            rhs=h0_sb[k],
                            start=(k == 0),
                            stop=(k == K1 - 1),
                        )
                    h = apool.tile([128, SC], F32R, tag=f"h1_{m}", name=f"h1_{c}_{m}")
                    nc.scalar.activation(h, ps, Relu, bias=b1_sb[:, m:m + 1])
                    h1_sb.append(h)

                for mt in range(MT):
                    ps = ppool2.tile([128, DOUT], F32, tag="ps2", name=f"ps2_{c}_{mt}")
                    for k in range(K1):
                        nc.tensor.matmul(
                            ps,
                            lhsT=h1_sb[k][:, mt * 128:(mt + 1) * 128],
                            rhs=w2_sb[k],
                            start=(k == 0),
                            stop=(k == K1 - 1),
                        )
                    ot = opool.tile([128, DOUT], F32, tag="ot", name=f"ot_{c}_{mt}")
                    nc.vector.tensor_add(ot, ps, b2_sb)
                    eng = nc.scalar if mt % 2 == 0 else nc.sync
                    eng.dma_start(
                        out=out[s0 + mt * 128:s0 + (mt + 1) * 128, :], in_=ot
                    )

            # Software pipeline: emit L0 of chunk c+1 ahead of L1/L2 of
            # chunk c, so the matmul stream never depends on a DMA issued
            # less than a full chunk earlier.
            h0_cur = layer0(0, q0_sb)
            for c in range(NCH):
                h0_next = None
                if c + 1 < NCH:
                    q_sb = q1_sb if c + 1 == 1 else load_q(c + 1)
                    h0_next = layer0(c + 1, q_sb)
                layers12(c, h0_cur)
                h0_cur = h0_next
    nc.finalize()
    return nc


_NC = None


def _get_nc():
    global _NC
    if _NC is None:
        _NC = build_nc()
    return _NC


def make_in_maps(inputs):
    bf16 = ml_dtypes.bfloat16
    q, W0, b0, W1, b1, W2, b2 = (
        inputs["query"], inputs["W0"], inputs["b0"], inputs["W1"],
        inputs["b1"], inputs["W2"], inputs["b2"],
    )
    in_maps = []
    for b in range(B):
        in_maps.append({
            "qT": np.ascontiguousarray(np.asarray(q[b]).T, dtype=np.float32),
            "w0t": np.ascontiguousarray(np.asarray(W0[b]).T, dtype=np.float32),
            "w1t": np.ascontiguousarray(np.asarray(W1[b]).T, dtype=np.float32),
            "w2t": np.ascontiguousarray(np.asarray(W2[b]).T, dtype=np.float32),
            "b0": np.asarray(b0[b], dtype=np.float32),
            "b1": np.asarray(b1[b], dtype=np.float32),
            "b2": np.asarray(b2[b], dtype=np.float32),
        })
    return in_maps


def run(inputs, trace=False):
    nc = _get_nc()
    in_maps = make_in_maps(inputs)
    res = run_bass_kernel_spmd(nc, in_maps, core_ids=list(range(B)), trace=trace)
    out = np.stack([np.asarray(r["out"], dtype=np.float32) for r in res.results])
    return out, res


def kernel(**inputs) -> np.ndarray:
    out, _ = run(inputs, trace=False)
    return out



# revision 4
# speedup vs baseline: 1.1222x; 1.1222x over previous
"""Trainium2 Bass kernel for nn_LongTermMemoryMLP.

Per-batch-weight 3-layer MLP:
    h0 = relu(q @ W0^T + b0); h1 = relu(h0 @ W1^T + b1); out = h1 @ W2^T + b2
with q: [B,S,DIN], W0: [B,DH,DIN], W1: [B,DH,DH], W2: [B,DOUT,DH], B=8.

Sharding: data-parallel over batch — one batch sample (and its weight slabs)
per NeuronCore, 8 cores, no cross-core communication.

Device-side strategy: activations are kept feature-major ([feature, seq],
feature on partitions) so every layer is a plain accumulated matmul with the
(pre-transposed) weights as the stationary operand and the activations as the
moving operand — no on-chip transposes. The final layer flips orientation
(stationary = activation tile, moving = W2^T) so the output lands seq-major
and can be DMA'd out contiguously. Inputs are pre-transposed AND pre-cast to
bf16 on the host: bf16 streams at the same 1 row/cycle as float32r but
enables the compiler's Fast Weight Load path (4 elem/cycle LDWEIGHTS, hidden
behind the previous matmul by the PE's reorder window) and halves all input
DMA traffic, which is what bounds the startup ramp. Accumulation stays fp32
in PSUM; measured end-to-end relative error is ~4e-3.
"""

import numpy as np

import ml_dtypes

import concourse.bass as bass
import concourse.tile as tile
from concourse import bacc, mybir
from concourse.bass_utils import run_bass_kernel_spmd

B, S, DIN, DH, DOUT = 8, 4096, 512, 1024, 512
SC = 512  # seq chunk processed per pipeline iteration

BF16 = mybir.dt.bfloat16
F32 = mybir.dt.float32


def build_nc():
    nc = bacc.Bacc("TRN2")
    qT = nc.dram_tensor("qT", (DIN, S), BF16, kind="ExternalInput")
    w0t = nc.dram_tensor("w0t", (DIN, DH), BF16, kind="ExternalInput")
    w1t = nc.dram_tensor("w1t", (DH, DH), BF16, kind="ExternalInput")
    w2t = nc.dram_tensor("w2t", (DH, DOUT), BF16, kind="ExternalInput")
    b0 = nc.dram_tensor("b0", (DH,), F32, kind="ExternalInput")
    b1 = nc.dram_tensor("b1", (DH,), F32, kind="ExternalInput")
    b2 = nc.dram_tensor("b2", (DOUT,), F32, kind="ExternalInput")
    out = nc.dram_tensor("out", (S, DOUT), F32, kind="ExternalOutput")

    K0 = DIN // 128   # 4  k-tiles, layer 0
    K1 = DH // 128    # 8  k-tiles, layers 1/2
    M0 = DH // 128    # 8  m-tiles (feature tiles of h0/h1)
    MT = SC // 128    # 4  seq m-tiles per chunk, layer 2
    NCH = S // SC     # 8  chunks

    Relu = mybir.ActivationFunctionType.Relu

    with tile.TileContext(nc) as tc:
        with (
            tc.tile_pool(name="weights", bufs=1) as wpool,
            tc.tile_pool(name="biases", bufs=1) as bpool,
            tc.tile_pool(name="acts", bufs=2) as apool,
            tc.tile_pool(name="qin", bufs=2) as qpool,
            tc.tile_pool(name="outp", bufs=4) as opool,
            tc.tile_pool(name="psum0", bufs=2, space="PSUM") as ppool0,
            tc.tile_pool(name="psum1", bufs=3, space="PSUM") as ppool1,
            tc.tile_pool(name="psum2", bufs=3, space="PSUM") as ppool2,
        ):
            # Pre-warm the PE clock gate (HAM) with dummy matmuls on garbage
            # data while the startup DMAs land: the real matmul stream then
            # starts at 2.4 GHz.
            g_lhs = apool.tile([128, 128], BF16, tag="warm_lhs")
            g_rhs = apool.tile([128, SC], BF16, tag="warm_rhs")
            nc.vector.memset(g_lhs, 0.0)
            nc.vector.memset(g_rhs, 0.0)
            warm_ps = ppool0.tile([128, SC], F32, tag="ps0")
            N_WARM = 12
            for i in range(N_WARM):
                nc.tensor.matmul(
                    warm_ps, lhsT=g_lhs, rhs=g_rhs,
                    start=(i == 0), stop=(i == N_WARM - 1),
                )

            # Startup loads, spread over the three DMA-issuing engines
            # (sync/scalar HWDGE rings + gpsimd SWDGE), first-chunk operands
            # first — all rings share the ~358 GB/s per-core HBM port:
            #   sync:   q0_0..3 | q1_0..3       (then steady-state q loads)
            #   scalar: w0_0 w0_1 | w1_0..3 | w2_0..3
            #   gpsimd: b0 b1 | w0_2 w0_3 | w1_4..7 | w2_4..7 | b2
            w0_sb = [wpool.tile([128, DH], BF16, tag=f"w0_{k}", name=f"w0_{k}") for k in range(K0)]
            q0_sb = [qpool.tile([128, SC], BF16, tag=f"q_{k}", name=f"q0_{k}") for k in range(K0)]
            b0_sb = bpool.tile([128, M0], F32, tag="b0")
            b1_sb = bpool.tile([128, M0], F32, tag="b1")
            nc.gpsimd.dma_start(out=b0_sb, in_=b0[:].rearrange("(m p) -> p m", p=128))
            nc.gpsimd.dma_start(out=b1_sb, in_=b1[:].rearrange("(m p) -> p m", p=128))
            for k in range(K0):
                weng = nc.scalar if k < 2 else nc.gpsimd
                weng.dma_start(out=w0_sb[k], in_=w0t[k * 128:(k + 1) * 128, :])
                nc.sync.dma_start(out=q0_sb[k], in_=qT[k * 128:(k + 1) * 128, 0:SC])

            q1_sb = []
            for k in range(K0):
                t = qpool.tile([128, SC], BF16, tag=f"q_{k}", name=f"q1pre_{k}")
                nc.sync.dma_start(out=t, in_=qT[k * 128:(k + 1) * 128, SC:2 * SC])
                q1_sb.append(t)

            w1_sb = [wpool.tile([128, DH], BF16, tag=f"w1_{k}", name=f"w1_{k}") for k in range(K1)]
            for k in range(K1):
                eng = nc.scalar if k < 4 else nc.gpsimd
                eng.dma_start(out=w1_sb[k], in_=w1t[k * 128:(k + 1) * 128, :])

            w2_sb = [wpool.tile([128, DOUT], BF16, tag=f"w2_{k}", name=f"w2_{k}") for k in range(K1)]
            for k in range(K1):
                eng = nc.scalar if k < 4 else nc.gpsimd
                eng.dma_start(out=w2_sb[k], in_=w2t[k * 128:(k + 1) * 128, :])
            b2_sb = bpool.tile([128, DOUT], F32, tag="b2")
            b2_ap = b2[:]
            b2_bcast = bass.AP(
                tensor=b2_ap.tensor,
                offset=b2_ap.offset,
                ap=[[0, 128]] + [list(d) for d in b2_ap.ap],
            )
            nc.gpsimd.dma_start(out=b2_sb, in_=b2_bcast)

            def load_q(c):
                s0 = c * SC
                q_sb = []
                for k in range(K0):
                    t = qpool.tile([128, SC], BF16, tag=f"q_{k}", name=f"q{c}_{k}")
                    nc.sync.dma_start(
                        out=t, in_=qT[k * 128:(k + 1) * 128, s0:s0 + SC]
                    )
                    q_sb.append(t)
                return q_sb

            def layer0(c, q_sb):
                h0_sb = []
                for m in range(M0):
                    ps = ppool0.tile([128, SC], F32, tag="ps0", name=f"ps0_{c}_{m}")
                    for k in range(K0):
                        nc.tensor.matmul(
                            ps,
                            lhsT=w0_sb[k][:, m * 128:(m + 1) * 128],
                            rhs=q_sb[k],
                            start=(k == 0),
                            stop=(k == K0 - 1),
                        )
                    h = apool.tile([128, SC], BF16, tag=f"h0_{m}", name=f"h0_{c}_{m}")
                    nc.scalar.activation(h, ps, Relu, bias=b0_sb[:, m:m + 1])
                    h0_sb.append(h)
                return h0_sb

            def layers12(c, h0_sb):
                s0 = c * SC
                h1_sb = []
                for m in range(M0):
                    ps = ppool1.tile([128, SC], F32, tag="ps1", name=f"ps1_{c}_{m}")
                    for k in range(K1):
                        nc.tensor.matmul(
                            ps,
                            lhsT=w1_sb[k][:, m * 128:(m + 1) * 128],
                            rhs=h0_sb[k],
                            start=(k == 0),
                            stop=(k == K1 - 1),
                        )
                    h = apool.tile([128, SC], BF16, tag=f"h1_{m}", name=f"h1_{c}_{m}")
                    nc.scalar.activation(h, ps, Relu, bias=b1_sb[:, m:m + 1])
                    h1_sb.append(h)

                for mt in range(MT):
                    ps = ppool2.tile([128, DOUT], F32, tag="ps2", name=f"ps2_{c}_{mt}")
                    for k in range(K1):
                        nc.tensor.matmul(
                            ps,
                            lhsT=h1_sb[k][:, mt * 128:(mt + 1) * 128],
                            rhs=w2_sb[k],
                            start=(k == 0),
                            stop=(k == K1 - 1),
                        )
                    ot = opool.tile([128, DOUT], F32, tag="ot", name=f"ot_{c}_{mt}")
                    nc.vector.tensor_add(ot, ps, b2_sb)
                    eng = nc.scalar if mt % 2 == 0 else nc.sync
                    eng.dma_start(
                        out=out[s0 + mt * 128:s0 + (mt + 1) * 128, :], in_=ot
                    )

            # Software pipeline: emit L0 of chunk c+1 ahead of L1/L2 of
            # chunk c, so the matmul stream never depends on a DMA issued
            # less than a full chunk earlier.
            h0_cur = layer0(0, q0_sb)
            for c in range(NCH):
                h0_next = None
                if c + 1 < NCH:
                    q_sb = q1_sb if c + 1 == 1 else load_q(c + 1)
                    h0_next = layer0(c + 1, q_sb)
                layers12(c, h0_cur)
                h0_cur = h0_next
    nc.finalize()
    return nc


_NC = None


def _get_nc():
    global _NC
    if _NC is None:
        _NC = build_nc()
    return _NC


def make_in_maps(inputs):
    bf16 = ml_dtypes.bfloat16
    q, W0, b0, W1, b1, W2, b2 = (
        inputs["query"], inputs["W0"], inputs["b0"], inputs["W1"],
        inputs["b1"], inputs["W2"], inputs["b2"],
    )
    in_maps = []
    for b in range(B):
        in_maps.append({
            "qT": np.ascontiguousarray(np.asarray(q[b]).T.astype(bf16)),
            "w0t": np.ascontiguousarray(np.asarray(W0[b]).T.astype(bf16)),
            "w1t": np.ascontiguousarray(np.asarray(W1[b]).T.astype(bf16)),
            "w2t": np.ascontiguousarray(np.asarray(W2[b]).T.astype(bf16)),
            "b0": np.asarray(b0[b], dtype=np.float32),
            "b1": np.asarray(b1[b], dtype=np.float32),
            "b2": np.asarray(b2[b], dtype=np.float32),
        })
    return in_maps


def run(inputs, trace=False):
    nc = _get_nc()
    in_maps = make_in_maps(inputs)
    res = run_bass_kernel_spmd(nc, in_maps, core_ids=list(range(B)), trace=trace)
    out = np.stack([np.asarray(r["out"], dtype=np.float32) for r in res.results])
    return out, res


def kernel(**inputs) -> np.ndarray:
    out, _ = run(inputs, trace=False)
    return out


# revision 5
# speedup vs baseline: 1.1402x; 1.0161x over previous
"""Trainium2 Bass kernel for nn_LongTermMemoryMLP.

Per-batch-weight 3-layer MLP:
    h0 = relu(q @ W0^T + b0); h1 = relu(h0 @ W1^T + b1); out = h1 @ W2^T + b2
with q: [B,S,DIN], W0: [B,DH,DIN], W1: [B,DH,DH], W2: [B,DOUT,DH], B=8.

Sharding: data-parallel over batch — one batch sample (and its weight slabs)
per NeuronCore, 8 cores, no cross-core communication.

Device-side strategy: activations are kept feature-major ([feature, seq],
feature on partitions) so every layer is a plain accumulated matmul with the
(pre-transposed) weights as the stationary operand and the activations as the
moving operand — no on-chip transposes. The final layer flips orientation
(stationary = activation tile, moving = W2^T) so the output lands seq-major
and can be DMA'd out contiguously. Inputs are pre-transposed AND pre-cast to
bf16 on the host: bf16 streams at the PE's full 1 row/cycle (518 cycles
measured per 128x128x512 matmul, the warm roofline) and halves all input DMA
traffic, which bounds the startup ramp. Weights and each seq-chunk of the
query load as single ~0.5-1 MiB DMAs (small transfers run at <50% DMA
efficiency; ~1 MiB runs at ~80%). Accumulation stays fp32 in PSUM; measured
end-to-end relative error is ~4e-3 against the fp32 reference.
"""

import numpy as np

import ml_dtypes

import concourse.bass as bass
import concourse.tile as tile
from concourse import bacc, mybir
from concourse.bass_utils import run_bass_kernel_spmd

B, S, DIN, DH, DOUT = 8, 4096, 512, 1024, 512
SC = 512  # seq chunk processed per pipeline iteration

BF16 = mybir.dt.bfloat16
F32 = mybir.dt.float32


def build_nc():
    nc = bacc.Bacc("TRN2")
    qT = nc.dram_tensor("qT", (DIN, S), BF16, kind="ExternalInput")
    w0t = nc.dram_tensor("w0t", (DIN, DH), BF16, kind="ExternalInput")
    w1t = nc.dram_tensor("w1t", (DH, DH), BF16, kind="ExternalInput")
    w2t = nc.dram_tensor("w2t", (DH, DOUT), BF16, kind="ExternalInput")
    b0 = nc.dram_tensor("b0", (DH,), F32, kind="ExternalInput")
    b1 = nc.dram_tensor("b1", (DH,), F32, kind="ExternalInput")
    b2 = nc.dram_tensor("b2", (DOUT,), F32, kind="ExternalInput")
    out = nc.dram_tensor("out", (S, DOUT), F32, kind="ExternalOutput")

    K0 = DIN // 128   # 4  k-tiles, layer 0
    K1 = DH // 128    # 8  k-tiles, layers 1/2
    M0 = DH // 128    # 8  m-tiles (feature tiles of h0/h1)
    MT = SC // 128    # 4  seq m-tiles per chunk, layer 2
    NCH = S // SC     # 8  chunks

    Relu = mybir.ActivationFunctionType.Relu

    with tile.TileContext(nc) as tc:
        with (
            tc.tile_pool(name="weights", bufs=1) as wpool,
            tc.tile_pool(name="biases", bufs=1) as bpool,
            tc.tile_pool(name="acts", bufs=2) as apool,
            tc.tile_pool(name="qin", bufs=2) as qpool,
            tc.tile_pool(name="outp", bufs=4) as opool,
            tc.tile_pool(name="psum0", bufs=2, space="PSUM") as ppool0,
            tc.tile_pool(name="psum1", bufs=3, space="PSUM") as ppool1,
            tc.tile_pool(name="psum2", bufs=3, space="PSUM") as ppool2,
        ):
            # Pre-warm the PE clock gate (HAM) with dummy matmuls on garbage
            # data while the startup DMAs land: the real matmul stream then
            # starts at 2.4 GHz.
            g_lhs = apool.tile([128, 128], BF16, tag="warm_lhs")
            g_rhs = apool.tile([128, SC], BF16, tag="warm_rhs")
            nc.vector.memset(g_lhs, 0.0)
            nc.vector.memset(g_rhs, 0.0)
            warm_ps = ppool0.tile([128, SC], F32, tag="ps0")
            N_WARM = 12
            for i in range(N_WARM):
                nc.tensor.matmul(
                    warm_ps, lhsT=g_lhs, rhs=g_rhs,
                    start=(i == 0), stop=(i == N_WARM - 1),
                )

            # Startup loads, spread over the three DMA-issuing engines
            # (sync/scalar HWDGE rings + gpsimd SWDGE), first-chunk operands
            # first, each as one big DMA:
            #   sync:   q(c0) | q(c1)            (then steady-state q loads)
            #   scalar: w0 | w1[k<4] | w2
            #   gpsimd: b0 b1 | w1[k>=4] | b2
            # Weight k-tiles live as the middle dim of one 3D SBUF tile.
            w0_sb = wpool.tile([128, K0, DH], BF16, tag="w0")
            b0_sb = bpool.tile([128, M0], F32, tag="b0")
            b1_sb = bpool.tile([128, M0], F32, tag="b1")
            nc.gpsimd.dma_start(out=b0_sb, in_=b0[:].rearrange("(m p) -> p m", p=128))
            nc.gpsimd.dma_start(out=b1_sb, in_=b1[:].rearrange("(m p) -> p m", p=128))
            nc.scalar.dma_start(
                out=w0_sb, in_=w0t[:, :].rearrange("(k p) h -> p k h", p=128)
            )

            def load_q(c, eng=None):
                s0 = c * SC
                t = qpool.tile([128, K0, SC], BF16, tag="q", name=f"q{c}")
                (eng or nc.sync).dma_start(
                    out=t, in_=qT[:, s0:s0 + SC].rearrange("(k p) s -> p k s", p=128)
                )
                return t

            q0_sb = load_q(0)
            q1_sb = load_q(1)

            w1a_sb = wpool.tile([128, K1 // 2, DH], BF16, tag="w1a")
            w1b_sb = wpool.tile([128, K1 // 2, DH], BF16, tag="w1b")
            nc.scalar.dma_start(
                out=w1a_sb, in_=w1t[0:DH // 2, :].rearrange("(k p) h -> p k h", p=128)
            )
            nc.gpsimd.dma_start(
                out=w1b_sb, in_=w1t[DH // 2:DH, :].rearrange("(k p) h -> p k h", p=128)
            )

            def w1_slice(k, m):
                t = w1a_sb if k < K1 // 2 else w1b_sb
                return t[:, k % (K1 // 2), m * 128:(m + 1) * 128]

            w2_sb = wpool.tile([128, K1, DOUT], BF16, tag="w2")
            nc.scalar.dma_start(
                out=w2_sb, in_=w2t[:, :].rearrange("(k p) o -> p k o", p=128)
            )
            b2_sb = bpool.tile([128, DOUT], F32, tag="b2")
            b2_ap = b2[:]
            b2_bcast = bass.AP(
                tensor=b2_ap.tensor,
                offset=b2_ap.offset,
                ap=[[0, 128]] + [list(d) for d in b2_ap.ap],
            )
            nc.gpsimd.dma_start(out=b2_sb, in_=b2_bcast)

            def layer0(c, q_sb):
                h0_sb = []
                for m in range(M0):
                    ps = ppool0.tile([128, SC], F32, tag="ps0", name=f"ps0_{c}_{m}")
                    for k in range(K0):
                        nc.tensor.matmul(
                            ps,
                            lhsT=w0_sb[:, k, m * 128:(m + 1) * 128],
                            rhs=q_sb[:, k, :],
                            start=(k == 0),
                            stop=(k == K0 - 1),
                        )
                    h = apool.tile([128, SC], BF16, tag=f"h0_{m}", name=f"h0_{c}_{m}")
                    nc.scalar.activation(h, ps, Relu, bias=b0_sb[:, m:m + 1])
                    h0_sb.append(h)
                return h0_sb

            def layers12(c, h0_sb):
                s0 = c * SC
                last = c == NCH - 1
                h1_sb = []
                for m in range(M0):
                    ps = ppool1.tile([128, SC], F32, tag="ps1", name=f"ps1_{c}_{m}")
                    for k in range(K1):
                        nc.tensor.matmul(
                            ps,
                            lhsT=w1_slice(k, m),
                            rhs=h0_sb[k],
                            start=(k == 0),
                            stop=(k == K1 - 1),
                        )
                    h = apool.tile([128, SC], BF16, tag=f"h1_{m}", name=f"h1_{c}_{m}")
                    nc.scalar.activation(h, ps, Relu, bias=b1_sb[:, m:m + 1])
                    h1_sb.append(h)

                for mt in range(MT):
                    ps = ppool2.tile([128, DOUT], F32, tag="ps2", name=f"ps2_{c}_{mt}")
                    for k in range(K1):
                        nc.tensor.matmul(
                            ps,
                            lhsT=h1_sb[k][:, mt * 128:(mt + 1) * 128],
                            rhs=w2_sb[:, k, :],
                            start=(k == 0),
                            stop=(k == K1 - 1),
                        )
                    ot = opool.tile([128, DOUT], F32, tag="ot", name=f"ot_{c}_{mt}")
                    r0 = s0 + mt * 128
                    if last and mt == MT - 1:
                        # Tail trim: halve the strictly-serial PSUM->add->DMA
                        # chain after the very last matmul.
                        H = DOUT // 2
                        nc.vector.tensor_add(ot[:, 0:H], ps[:, 0:H], b2_sb[:, 0:H])
                        nc.scalar.dma_start(
                            out=out[r0:r0 + 128, 0:H], in_=ot[:, 0:H]
                        )
                        nc.vector.tensor_add(ot[:, H:], ps[:, H:], b2_sb[:, H:])
                        nc.sync.dma_start(out=out[r0:r0 + 128, H:], in_=ot[:, H:])
                    else:
                        nc.vector.tensor_add(ot, ps, b2_sb)
                        eng = nc.scalar if mt % 2 == 0 else nc.sync
                        eng.dma_start(out=out[r0:r0 + 128, :], in_=ot)

            # Software pipeline: emit L0 of chunk c+1 ahead of L1/L2 of
            # chunk c, so the matmul stream never depends on a DMA issued
            # less than a full chunk earlier.
            h0_cur = layer0(0, q0_sb)
            for c in range(NCH):
                h0_next = None
                if c + 1 < NCH:
                    q_sb = q1_sb if c + 1 == 1 else load_q(c + 1)
                    h0_next = layer0(c + 1, q_sb)
                layers12(c, h0_cur)
                h0_cur = h0_next
    nc.finalize()
    return nc


_NC = None


def _get_nc():
    global _NC
    if _NC is None:
        _NC = build_nc()
    return _NC


def make_in_maps(inputs):
    bf16 = ml_dtypes.bfloat16
    q, W0, b0, W1, b1, W2, b2 = (
        inputs["query"], inputs["W0"], inputs["b0"], inputs["W1"],
        inputs["b1"], inputs["W2"], inputs["b2"],
    )
    in_maps = []
    for b in range(B):
        in_maps.append({
            "qT": np.ascontiguousarray(np.asarray(q[b]).T.astype(bf16)),
            "w0t": np.ascontiguousarray(np.asarray(W0[b]).T.astype(bf16)),
            "w1t": np.ascontiguousarray(np.asarray(W1[b]).T.astype(bf16)),
            "w2t": np.ascontiguousarray(np.asarray(W2[b]).T.astype(bf16)),
            "b0": np.asarray(b0[b], dtype=np.float32),
            "b1": np.asarray(b1[b], dtype=np.float32),
            "b2": np.asarray(b2[b], dtype=np.float32),
        })
    return in_maps


def run(inputs, trace=False):
    nc = _get_nc()
    in_maps = make_in_maps(inputs)
    res = run_bass_kernel_spmd(nc, in_maps, core_ids=list(range(B)), trace=trace)
    out = np.stack([np.asarray(r["out"], dtype=np.float32) for r in res.results])
    return out, res


def kernel(**inputs) -> np.ndarray:
    out, _ = run(inputs, trace=False)
    return out
